# revision 24
# baseline (speedup 1.0000x reference)
"""Trainium2 Bass kernel for nn_CrossAttention (GQA cross-attention + RMSNorm + residual).

Sharding: 8 cores = (batch b in {0,1}) x (kv-head group g in {0..3}).
Each core computes, for its (b, g): the R=4 query heads of group g over the
full sequence, producing a partial output y_bg = attn_out_g @ wo_g^T (the
g-slice columns of wo). Host gathers: out[b] = x[b] + sum_g y_bg.

Device-side layout is fully transposed ([feature, seq]) so every matmul
contracts over the partition dim. RMSNorm gains are folded into the weights
on host; per-row rstd factors are folded into qT (DVE bcast multiply), the
exp() scale argument (rstd_kv is per-partition in scoresT layout), and v.
Softmax runs in scoresT [t, s] layout: exp on ACT, sums via ones-matmul,
division folded into a DVE psum->sbuf copy before the wo projection.

All matmul operands are bf16 (cast on host for DMA-fed tensors); PSUM and
softmax statistics stay fp32.
"""

import os

import numpy as np
import ml_dtypes

import concourse.bass as bass
import concourse.mybir as mybir
import concourse.tile as tile
from concourse import bacc
from concourse.bass import ts
from concourse.bass_utils import run_bass_kernel_spmd
from concourse.masks import make_identity

F32 = mybir.dt.float32
BF16 = mybir.dt.bfloat16
BF = ml_dtypes.bfloat16
AF = mybir.ActivationFunctionType

B, S, T, D = 2, 2048, 2048, 1024
H, HKV, HD = 16, 4, 64
R = H // HKV            # 4 query heads per kv group (per core)
E = R * HD              # 256: per-core q / attn-out feature width
DB = D // 128           # 8 d-blocks
NTB = T // 128          # 16 t-blocks
STW = 512               # s-tile width
NST = S // STW          # 4 s-tiles
EPS = 1e-5

LAST_RESULTS = None     # BassKernelResults of the most recent run (for test.py)


def _pbcast(ap, parts):
    """[1, N] AP -> [parts, N] partition-broadcast AP (stride-0 partition dim)."""
    assert ap.shape[0] == 1
    return bass.AP(tensor=ap.tensor, offset=ap.offset, ap=[[0, parts]] + list(ap.ap[1:]))


def build_kernel():
    nc = bacc.Bacc("TRN2", target_bir_lowering=False, debug=False)

    xT = nc.dram_tensor("xT", [D, S], BF16, kind="ExternalInput").ap()
    kvT = nc.dram_tensor("kvT", [D, T], BF16, kind="ExternalInput").ap()
    wqT = nc.dram_tensor("wqT", [D, E], BF16, kind="ExternalInput").ap()
    # columns 0-63 = wv_g, 64-127 = wk_g (v first; see kv phase partition bases)
    wkvT = nc.dram_tensor("wkvT", [D, 2 * HD], BF16, kind="ExternalInput").ap()
    woT = nc.dram_tensor("woT", [E, D], BF16, kind="ExternalInput").ap()
    y = nc.dram_tensor("y", [S, D], F32, kind="ExternalOutput").ap()

    with tile.TileContext(nc) as tc:
        _body(tc, xT, kvT, wqT, wkvT, woT, y)
    nc.finalize()
    return nc


def _body(tc, xT, kvT, wqT, wkvT, woT, y):
    nc = tc.nc
    mm = nc.tensor.matmul

    import contextlib
    ctx = contextlib.ExitStack()
    with ctx:
        persist = ctx.enter_context(tc.tile_pool(name="persist", bufs=1))
        sqpool = ctx.enter_context(tc.tile_pool(name="sq", bufs=4))
        small = ctx.enter_context(tc.tile_pool(name="small", bufs=2))
        dram = ctx.enter_context(tc.tile_pool(name="dram", bufs=2, space="DRAM"))

        # ---- constants ----
        ones_sb = persist.tile([128, 1], BF16)
        nc.vector.memset(ones_sb[:], 1.0)
        ident = persist.tile([128, 128], BF16)
        make_identity(nc, ident[:])
        eps_kv = persist.tile([128, 1], F32)
        nc.vector.memset(eps_kv[:], EPS)
        eps_x = persist.tile([1, 1], F32)
        nc.vector.memset(eps_x[:], 64.0 * EPS)

        # ---- full-tensor loads (transposed layouts, bf16) ----
        # order: kv-proj weights first, then interleaved kvT/xT d-blocks so
        # both sumsq/projection chains start as soon as their block lands
        kvT_r = kvT.rearrange("(o p) s -> p o s", p=128)
        xT_r = xT.rearrange("(o p) s -> p o s", p=128)
        wkv_sb = persist.tile([128, DB, 2 * HD], BF16)
        nc.sync.dma_start(wkv_sb[:], wkvT.rearrange("(o p) e -> p o e", p=128))
        kvT_sb = persist.tile([128, DB, T], BF16)
        xT_sb = persist.tile([128, DB, S], BF16)
        for db in range(DB):
            nc.sync.dma_start(kvT_sb[:, db, :], kvT_r[:, db, :])
            nc.sync.dma_start(xT_sb[:, db, :], xT_r[:, db, :])
        wq_sb = persist.tile([128, DB, E], BF16)
        nc.sync.dma_start(wq_sb[:], wqT.rearrange("(o p) e -> p o e", p=128))
        wo_sb = persist.tile([128, 2, D], BF16)
        nc.sync.dma_start(wo_sb[:], woT.rearrange("(o p) d -> p o d", p=128))

        # ---- persistent intermediates ----
        k2_sb = persist.tile([128, T], BF16)        # kT duplicated on both 64-row halves
        v_sb = persist.tile([128, NTB, HD], BF16)   # v * rstd_kv, [t-block, 128 x 64]
        q_sb = persist.tile([128, 2, S], BF16)      # qT * rstd_q/8, e-blocks on axis 1
        rkv_sb = persist.tile([128, NTB], F32)      # rstd_kv per t-block, per-partition
        rq_bcast = persist.tile([128, S], F32)      # rstd_q/8 broadcast over partitions

        # ========== prologue: kv/x sumsq + k/v/q projections (one psum pool) ==========
        # ss bank i: partition 64 = sumsq(kv) for t-tile i, partition 0 = sumsq(x)
        # for s-tile i. kv-proj and q-proj share the two "mm128" psum slots.
        with tc.tile_pool(name="pps", bufs=1, space="PSUM") as pps, \
             tc.tile_pool(name="vvp", bufs=2) as vvp:
            ss_ps = pps.tile([65, 4, STW], F32, tag="ss")
            for db in range(DB):
                sq = sqpool.tile([128, T], BF16, tag="sq")
                nc.vector.tensor_mul(sq[:], kvT_sb[:, db, :], kvT_sb[:, db, :])
                sqx = sqpool.tile([128, S], BF16, tag="sq")
                nc.vector.tensor_mul(sqx[:], xT_sb[:, db, :], xT_sb[:, db, :])
                # kv/x pairs adjacent: distinct col strips (64 vs 0) run
                # concurrently in the PE array
                for i in range(4):
                    mm(ss_ps[64:65, i, :], ones_sb[:, 0:1], sq[:, ts(i, STW)],
                       start=(db == 0), stop=(db == DB - 1),
                       tile_position=(0, 64), skip_group_check=True)
                    mm(ss_ps[0:1, i, :], ones_sb[:, 0:1], sqx[:, ts(i, STW)],
                       start=(db == 0), stop=(db == DB - 1),
                       skip_group_check=True)

            for tt in range(4):
                # one accumulation chain computes vT (rows 0-63) and kT (64-127)
                kvp = pps.tile([128, STW], F32, tag="mm128", bufs=2)
                for db in range(DB):
                    mm(kvp[:], wkv_sb[:, db, :], kvT_sb[:, db, ts(tt, STW)],
                       start=(db == 0), stop=(db == DB - 1))
                nc.vector.tensor_copy(k2_sb[64:128, ts(tt, STW)], kvp[64:128, :])
                vv = vvp.tile([65, STW], BF16, tag="vv")
                nc.vector.tensor_copy(vv[0:64, :], kvp[0:64, :])
                nc.vector.tensor_copy(vv[64:65, :], ss_ps[64:65, tt, :])

                for i in range(4):
                    tb = tt * 4 + i
                    tp = pps.tile([128, 65], BF16, tag="tp", bufs=2)
                    nc.tensor.transpose(tp[:], vv[:, ts(i, 128)], ident[0:65, 0:65])
                    tmp = small.tile([128, 1], F32, tag="sqv")
                    nc.scalar.activation(tmp[:], tp[:, 64:65], AF.Sqrt,
                                         scale=1.0 / 1024.0, bias=eps_kv[:, 0:1])
                    nc.vector.reciprocal(rkv_sb[:, tb:tb + 1], tmp[:])
                    nc.vector.tensor_scalar_mul(v_sb[:, tb, :], tp[:, 0:64],
                                                rkv_sb[:, tb:tb + 1])
            # duplicate kT onto partitions 0-63 (partition move => DMA)
            nc.sync.dma_start(k2_sb[0:64, :], k2_sb[64:128, :])

            # rstd_q/8 = 1/sqrt(64*ss/1024 + 64*eps), broadcast via DRAM roundtrip
            rqs = small.tile([1, S], F32, tag="rqs")
            nc.scalar.activation(rqs[:], ss_ps[0:1, :, :], AF.Sqrt,
                                 scale=0.0625, bias=eps_x[:, 0:1])
            rq_vec = small.tile([1, S], F32, tag="rqv")
            nc.vector.reciprocal(rq_vec[:], rqs[:])
            rq_dram = dram.tile([1, S], F32, bufs=1)
            nc.sync.dma_start(rq_dram[:], rq_vec[:])
            nc.sync.dma_start(rq_bcast[:], _pbcast(rq_dram[:], 128))

        # ================= Attention + output =================
        with tc.tile_pool(name="aps", bufs=1, space="PSUM") as aps, \
             tc.tile_pool(name="psb", bufs=4) as psb, \
             tc.tile_pool(name="asb", bufs=2) as asb, \
             tc.tile_pool(name="ypool", bufs=2) as ypool:
            # q projection shares the "misc" psum bank with the y projection;
            # only qT(st=0) gates the first QK matmuls, the rest hide under attn
            for st in range(NST):
                for eb in range(2):
                    qps = aps.tile([128, STW], F32, tag="misc", bufs=1)
                    for db in range(DB):
                        mm(qps[:], wq_sb[:, db, ts(eb, 128)], xT_sb[:, db, ts(st, STW)],
                           start=(db == 0), stop=(db == DB - 1))
                    nc.vector.tensor_mul(q_sb[:, eb, ts(st, STW)], qps[:],
                                         rq_bcast[:, ts(st, STW)])

            for st in range(NST):
                out_ps = aps.tile([128, 2, STW], F32, tag="out")
                sums_ps = aps.tile([128, STW], F32, tag="sums")
                for tb in range(NTB):
                    for grp in range(2):
                        sc = aps.tile([128, 2, STW], F32, tag="scores", bufs=2)
                        for hh in range(2):
                            mm(sc[:, hh, :],
                               k2_sb[64 * hh:64 * hh + 64, ts(tb, 128)],
                               q_sb[64 * hh:64 * hh + 64, grp, ts(st, STW)],
                               start=True, stop=True)
                        pT = psb.tile([128, 2, STW], BF16, tag="pT")
                        nc.scalar.activation(pT[:, :, :], sc[:, :, :], AF.Exp,
                                             scale=rkv_sb[:, tb:tb + 1])
                        for hh in range(2):
                            h = grp * 2 + hh
                            mm(out_ps[64 * hh:64 * hh + 64, grp, :],
                               v_sb[:, tb, :], pT[:, hh, :],
                               start=(tb == 0), stop=(tb == NTB - 1),
                               skip_group_check=True)
                            mm(sums_ps[32 * h:32 * h + 1, :],
                               ones_sb[:, 0:1], pT[:, hh, :],
                               start=(tb == 0), stop=(tb == NTB - 1),
                               tile_position=(0, 32 * h),
                               skip_group_check=True)

                # drain psum accumulators quickly so (st+1) matmuls can start,
                # then normalize off the critical path
                attn_raw = asb.tile([128, 2, STW], F32, tag="araw")
                for j in range(2):
                    nc.vector.tensor_copy(attn_raw[:, j, :], out_ps[:, j, :])
                recips = asb.tile([128, STW], F32, tag="recips")
                for h in range(4):
                    nc.vector.reciprocal(recips[32 * h:32 * h + 1, :],
                                         sums_ps[32 * h:32 * h + 1, :])
                rec_dram = dram.tile([4, STW], F32, tag="rec")
                for h in range(4):
                    nc.sync.dma_start(rec_dram[h:h + 1, :],
                                      recips[32 * h:32 * h + 1, :])
                rb = asb.tile([128, 2, STW], F32, tag="rb")
                for j in range(2):
                    for i in range(2):
                        h = 2 * j + i
                        nc.sync.dma_start(rb[64 * i:64 * i + 64, j, :],
                                          _pbcast(rec_dram[h:h + 1, :], 64))
                attn_sb = asb.tile([128, 2, STW], BF16, tag="attn")
                for j in range(2):
                    nc.vector.tensor_mul(attn_sb[:, j, :], attn_raw[:, j, :], rb[:, j, :])

                # y[s_block, :] = attn_sb[:, :, s_block].T @ wo
                for sb_i in range(4):
                    y_sb = ypool.tile([128, D], F32, tag="y")
                    for dt in range(2):
                        if st == NST - 1:
                            # last s-tile: attention is done, reuse the freed
                            # scores slots for a deeper y pipeline
                            yps2 = aps.tile([128, 2, STW], F32, tag="scores",
                                            bufs=2, name="yps2")
                            yps = yps2[:, 0, :]
                        else:
                            yps = aps.tile([128, STW], F32, tag="misc", bufs=1)
                        for j in range(2):
                            mm(yps[:], attn_sb[:, j, ts(sb_i, 128)],
                               wo_sb[:, j, ts(dt, STW)],
                               start=(j == 0), stop=(j == 1))
                        nc.vector.tensor_copy(y_sb[:, ts(dt, STW)], yps[:])
                    nc.sync.dma_start(y[st * STW + sb_i * 128:st * STW + sb_i * 128 + 128, :],
                                      y_sb[:])


_NC_CACHE = None


def kernel(x, kv, wq, wk, wv, wo, gq, gkv):
    global LAST_RESULTS, _NC_CACHE
    x = np.asarray(x, dtype=np.float32)
    kv = np.asarray(kv, dtype=np.float32)
    wq = np.asarray(wq, dtype=np.float32)
    wk = np.asarray(wk, dtype=np.float32)
    wv = np.asarray(wv, dtype=np.float32)
    wo = np.asarray(wo, dtype=np.float32)
    gq = np.asarray(gq, dtype=np.float32)
    gkv = np.asarray(gkv, dtype=np.float32)

    # fold RMSNorm gains into the projection weights
    wq_f = wq * gq[None, :]
    wk_f = wk * gkv[None, :]
    wv_f = wv * gkv[None, :]

    def c(a):
        return np.ascontiguousarray(a.astype(BF))

    in_maps = []
    for core in range(8):
        b, g = divmod(core, HKV)
        wkv_g = np.concatenate([wv_f[g * HD:(g + 1) * HD, :].T,
                                wk_f[g * HD:(g + 1) * HD, :].T], axis=1)
        in_maps.append({
            "xT": c(x[b].T),
            "kvT": c(kv[b].T),
            "wqT": c(wq_f[g * E:(g + 1) * E, :].T),
            "wkvT": c(wkv_g),
            "woT": c(wo[:, g * E:(g + 1) * E].T),
        })

    if _NC_CACHE is None:
        _NC_CACHE = build_kernel()
    nc = _NC_CACHE

    trace = os.environ.get("KERNEL_TRACE", "0") == "1"
    try:
        res = run_bass_kernel_spmd(nc, in_maps, core_ids=list(range(8)), trace=trace)
    except ModuleNotFoundError:
        # NTFF profiling hook unavailable in this container; run untraced
        res = run_bass_kernel_spmd(nc, in_maps, core_ids=list(range(8)), trace=False)
    LAST_RESULTS = res

    out = np.empty((B, S, D), np.float32)
    for b in range(B):
        acc = x[b].copy()
        for g in range(HKV):
            acc += res.results[b * HKV + g]["y"]
        out[b] = acc
    return out
